# revision 3
# baseline (speedup 1.0000x reference)
"""EntityAttention Trainium2 kernel (nn_EntityAttention_31525059952740).

Math (per (batch, entity) group n, all 64 events e):
  q = (events @ Wq.T + bq) * scale            shared across n     [64, 512]
  k = toks_b @ Wk.T + bk                      per batch           [512, 512]
  v = toks_b @ Wv.T + bv                      per batch           [512, 512]
  scores[h,e,s] = q_h[e] . k_h[s]             per batch (2 heads x 256)
  attn = softmax over s, masked by entities[n]  (mask = multiplicative
         0/1 on exp since exp(-1e9 + x) == 0 in fp32)
  out[e] = concat_h(attn_h @ v_h);  O = out @ Wo.T + bo

Sharding: batch b -> core b (8 batches, 8 cores). Each core computes all 16
entities of its batch -> output rows [1024, 512] per core, concatenated.

Device layout ("transposed attention"): scores^T [s(partitions), (h,e)] so
that the entity mask is a per-partition scalar (fused into a single DVE
tensor_scalar per (entity, s-chunk)) and softmax denominators come from one
tiny PE matmul  S = masks^T.T @ exp(scores^T).

All heavy matmuls run as float32r (full PE rate, ~1e-4 relative error).
"""

import numpy as np

import concourse.bass as bass
import concourse.tile as tile
import concourse.mybir as mybir
from concourse import bacc
from concourse.bass_utils import run_bass_kernel_spmd
from concourse.masks import make_identity

NB, SL, NH, EN, NE, HEADS = 8, 512, 512, 16, 64, 2
DH = NH // HEADS          # 256
P = 128
NCHUNK = NH // P          # 4 chunks of the hidden dim
SCHUNK = SL // P          # 4 chunks of the sequence dim
SCALE = 1.0 / np.sqrt(DH).astype(np.float32)

F32 = mybir.dt.float32
F32R = mybir.dt.float32r

_CACHE = {}


def _build():
    nc = bacc.Bacc("TRN2", target_bir_lowering=False, debug=False, num_devices=NB)

    # ---- I/O ----
    toks_d = nc.dram_tensor("toks", [SL, NH], F32, kind="ExternalInput").ap()
    masks_d = nc.dram_tensor("masksT", [P, SCHUNK, EN], F32R, kind="ExternalInput").ap()
    evT_d = nc.dram_tensor("eventsT", [NH, NE], F32R, kind="ExternalInput").ap()
    wqT_d = nc.dram_tensor("WqT", [NH, NH], F32R, kind="ExternalInput").ap()
    wkT_d = nc.dram_tensor("WkT", [NH, NH], F32R, kind="ExternalInput").ap()
    wvT_d = nc.dram_tensor("WvT", [NH, NH], F32R, kind="ExternalInput").ap()
    woT_d = nc.dram_tensor("WoT", [NH, NH], F32R, kind="ExternalInput").ap()
    bq_d = nc.dram_tensor("bq", [NH], F32, kind="ExternalInput").ap()
    bk_d = nc.dram_tensor("bk_pc", [P, NCHUNK], F32, kind="ExternalInput").ap()
    bv_d = nc.dram_tensor("bv", [NH], F32, kind="ExternalInput").ap()
    bo_d = nc.dram_tensor("bo", [NH], F32, kind="ExternalInput").ap()
    out_d = nc.dram_tensor("out", [EN * NE, NH], F32, kind="ExternalOutput").ap()

    with tile.TileContext(nc) as tc:
        with (
            tc.tile_pool(name="wpool", bufs=1) as wpool,
            tc.tile_pool(name="sb", bufs=1) as sb,
            tc.tile_pool(name="ostage", bufs=3) as ostage,
            tc.tile_pool(name="pbig", bufs=3, space="PSUM") as pbig,
            tc.tile_pool(name="psmall", bufs=2, space="PSUM") as psmall,
            tc.tile_pool(name="pq", bufs=1, space="PSUM") as pq,
            tc.tile_pool(name="dram", bufs=1, space="DRAM") as dram,
        ):
            # ---------- loads ----------
            toks_sb = sb.tile([P, SCHUNK, NH], F32, tag="toks")
            nc.sync.dma_start(toks_sb[:], toks_d.rearrange("(sc p) h -> p sc h", p=P))

            masks_sb = sb.tile([P, SCHUNK, EN], F32R, tag="masks")
            nc.sync.dma_start(masks_sb[:], masks_d)

            wk_sb = wpool.tile([P, NCHUNK, NH], F32R, tag="wk")
            nc.sync.dma_start(wk_sb[:], wkT_d.rearrange("(c p) d -> p c d", p=P))
            wv_sb = wpool.tile([P, NCHUNK, NH], F32R, tag="wv")
            nc.sync.dma_start(wv_sb[:], wvT_d.rearrange("(c p) d -> p c d", p=P))
            wq_sb = wpool.tile([P, NCHUNK, NH], F32R, tag="wq")
            nc.sync.dma_start(wq_sb[:], wqT_d.rearrange("(c p) d -> p c d", p=P))
            wo_sb = wpool.tile([P, NCHUNK, NH], F32R, tag="wo")
            nc.sync.dma_start(wo_sb[:], woT_d.rearrange("(c p) d -> p c d", p=P))
            ev_sb = wpool.tile([P, NCHUNK, NE], F32R, tag="ev")
            nc.sync.dma_start(ev_sb[:], evT_d.rearrange("(c p) e -> p c e", p=P))

            bk_sb = wpool.tile([P, NCHUNK], F32, tag="bk")
            nc.sync.dma_start(bk_sb[:], bk_d)

            # free-dim biases, replicated across all 128 partitions
            def bcast_load(name, src):
                t = wpool.tile([P, NH], F32, tag=name)
                src_b = bass.AP(tensor=src.tensor, offset=src.offset,
                                ap=[[0, P], *src.ap])
                nc.gpsimd.dma_start(out=t[:], in_=src_b)
                return t

            bq_bc = bcast_load("bq_bc", bq_d)
            bv_bc = bcast_load("bv_bc", bv_d)
            bo_bc = bcast_load("bo_bc", bo_d)

            ident = wpool.tile([P, P], F32, tag="ident")
            make_identity(nc, ident[:])

            # ---------- tokens transpose: toksT[hid, s] ----------
            toksT = sb.tile([P, NCHUNK, SL], F32R, tag="toksT")
            for hc in range(NCHUNK):
                ptt = pbig.tile([P, SL], F32, tag="pb")
                for sc in range(SCHUNK):
                    nc.tensor.transpose(
                        ptt[:, sc * P:(sc + 1) * P],
                        toks_sb[:, sc, hc * P:(hc + 1) * P],
                        ident[:],
                    )
                nc.scalar.activation(toksT[:, hc, :], ptt[:], mybir.ActivationFunctionType.Copy)

            # ---------- K^T = WkT.T @ toksT + bk  [dout, s] ----------
            kT = sb.tile([P, NCHUNK, SL], F32R, tag="kT")
            for dc in range(NCHUNK):
                pk = pbig.tile([P, SL], F32, tag="pb")
                for hc in range(NCHUNK):
                    nc.tensor.matmul(
                        pk[:], wk_sb[:, hc, dc * P:(dc + 1) * P], toksT[:, hc, :],
                        start=(hc == 0), stop=(hc == NCHUNK - 1),
                    )
                nc.scalar.activation(kT[:, dc, :], pk[:],
                                     mybir.ActivationFunctionType.Identity,
                                     bias=bk_sb[:, dc:dc + 1])

            # ---------- V = toks @ WvT + bv  [s, dout] ----------
            v_sb = sb.tile([P, SCHUNK, NH], F32R, tag="v")
            for sc in range(SCHUNK):
                pv = pbig.tile([P, NH], F32, tag="pb")
                for hc in range(NCHUNK):
                    nc.tensor.matmul(
                        pv[:], toksT[:, hc, sc * P:(sc + 1) * P], wv_sb[:, hc, :],
                        start=(hc == 0), stop=(hc == NCHUNK - 1),
                    )
                nc.vector.tensor_add(v_sb[:, sc, :], pv[:], bv_bc[:])

            # ---------- Q = events @ WqT + bq (scaled); then QT ----------
            pq_t = pq.tile([P, NH], F32, tag="pqq")
            for hc in range(NCHUNK):
                nc.tensor.matmul(
                    pq_t[:NE], ev_sb[:, hc, :], wq_sb[:, hc, :],
                    start=(hc == 0), stop=(hc == NCHUNK - 1),
                )
            q_sb = sb.tile([NE, NH], F32, tag="q")
            nc.vector.tensor_add(q_sb[:], pq_t[:NE], bq_bc[:NE])

            qT = sb.tile([P, NCHUNK, NE], F32R, tag="qT")
            for dc in range(NCHUNK):
                pqt = psmall.tile([P, NE], F32, tag="psm")
                nc.tensor.transpose(pqt[:], q_sb[:, dc * P:(dc + 1) * P],
                                    ident[:NE, :NE])
                nc.scalar.activation(qT[:, dc, :], pqt[:],
                                     mybir.ActivationFunctionType.Copy)

            # ---------- scores^T and E = exp(scores^T)  [s, (h,e)] ----------
            e_sb = sb.tile([P, SCHUNK, HEADS * NE], F32R, tag="e")
            for sc in range(SCHUNK):
                ps = psmall.tile([P, HEADS * NE], F32, tag="psm")
                for h in range(HEADS):
                    for j in range(2):
                        dc = 2 * h + j
                        nc.tensor.matmul(
                            ps[:, h * NE:(h + 1) * NE],
                            kT[:, dc, sc * P:(sc + 1) * P], qT[:, dc, :],
                            start=(j == 0), stop=(j == 1),
                        )
                nc.scalar.activation(e_sb[:, sc, :], ps[:],
                                     mybir.ActivationFunctionType.Exp)

            # ---------- S[ent, (h,e)] = masksT.T @ E ; recip; broadcast ----------
            pS = psmall.tile([EN, HEADS * NE], F32, tag="psm")
            for sc in range(SCHUNK):
                nc.tensor.matmul(pS[:], masks_sb[:, sc, :], e_sb[:, sc, :],
                                 start=(sc == 0), stop=(sc == SCHUNK - 1))
            srec = sb.tile([EN, HEADS * NE], F32, tag="srec")
            nc.vector.reciprocal(srec[:], pS[:])

            srec_dram = dram.tile([EN, HEADS * NE], F32)
            nc.sync.dma_start(srec_dram[:], srec[:])
            srec_bc = sb.tile([P, EN, HEADS * NE], F32, tag="srec_bc")
            sd_ap = srec_dram[:]
            nc.gpsimd.dma_start(
                out=srec_bc[:],
                in_=bass.AP(tensor=sd_ap.tensor, offset=sd_ap.offset,
                            ap=[[0, P], *sd_ap.ap]),
            )

            # ---------- attnT[s, ent, (h,e)] = E * mask (per-partition scalar) --
            attnT = sb.tile([P, SCHUNK, EN, HEADS * NE], F32R, tag="attnT")
            for ent in range(EN):
                for sc in range(SCHUNK):
                    eng = nc.vector if (ent % 2 == 0) else nc.scalar
                    if eng is nc.vector:
                        nc.vector.tensor_scalar_mul(
                            attnT[:, sc, ent, :], e_sb[:, sc, :],
                            masks_sb[:, sc, ent:ent + 1].bitcast(F32),
                        )
                    else:
                        nc.scalar.activation(
                            attnT[:, sc, ent, :], e_sb[:, sc, :],
                            mybir.ActivationFunctionType.Copy,
                            scale=masks_sb[:, sc, ent:ent + 1].bitcast(F32),
                        )

            # ---------- PV: outT[d, ent, e] = V_h.T @ attnT_h, normalized ------
            outT = sb.tile([P, NCHUNK, EN, NE], F32R, tag="outT")
            for h in range(HEADS):
                for j in range(2):
                    dc = 2 * h + j
                    for grp in range(2):
                        po = pbig.tile([P, 8 * NE], F32, tag="pb")
                        for sc in range(SCHUNK):
                            nc.tensor.matmul(
                                po[:],
                                v_sb[:, sc, dc * P:(dc + 1) * P],
                                attnT[:, sc, grp * 8:(grp + 1) * 8,
                                      h * NE:(h + 1) * NE],
                                start=(sc == 0), stop=(sc == SCHUNK - 1),
                            )
                        nc.vector.tensor_mul(
                            outT[:, dc, grp * 8:(grp + 1) * 8, :], po[:],
                            srec_bc[:, grp * 8:(grp + 1) * 8,
                                    h * NE:(h + 1) * NE],
                        )

            # ---------- O = outT.T @ WoT + bo  [(ent,e), dout] ----------
            for pair in range(EN // 2):
                pO = pbig.tile([P, NH], F32, tag="pb")
                for hc in range(NCHUNK):
                    nc.tensor.matmul(
                        pO[:], outT[:, hc, 2 * pair:2 * pair + 2, :],
                        wo_sb[:, hc, :],
                        start=(hc == 0), stop=(hc == NCHUNK - 1),
                    )
                o_sb = ostage.tile([P, NH], F32, tag="o_sb")
                nc.vector.tensor_add(o_sb[:], pO[:], bo_bc[:])
                nc.sync.dma_start(out_d[pair * P:(pair + 1) * P, :], o_sb[:])

    nc.compile()
    return nc


def _get_nc():
    if "nc" not in _CACHE:
        _CACHE["nc"] = _build()
    return _CACHE["nc"]


def kernel(tokens_embed, entities, events_embed, entity_num, entity_masks,
           select_event, Wq, Wk, Wv, bq, bk, bv, Wo, bo):
    tokens_embed = np.asarray(tokens_embed, dtype=np.float32)
    entities = np.asarray(entities)
    events_embed = np.asarray(events_embed, dtype=np.float32)
    entity_masks = np.asarray(entity_masks)
    select_event = np.asarray(select_event)
    Wq = np.asarray(Wq, dtype=np.float32)
    Wk = np.asarray(Wk, dtype=np.float32)
    Wv = np.asarray(Wv, dtype=np.float32)
    Wo = np.asarray(Wo, dtype=np.float32)
    bq = np.asarray(bq, dtype=np.float32)
    bk = np.asarray(bk, dtype=np.float32)
    bv = np.asarray(bv, dtype=np.float32)
    bo = np.asarray(bo, dtype=np.float32)

    nc = _get_nc()

    shared = {
        "eventsT": np.ascontiguousarray(events_embed.T),
        "WqT": np.ascontiguousarray((Wq * SCALE).T),
        "WkT": np.ascontiguousarray(Wk.T),
        "WvT": np.ascontiguousarray(Wv.T),
        "WoT": np.ascontiguousarray(Wo.T),
        "bq": np.ascontiguousarray(bq * SCALE),
        "bk_pc": np.ascontiguousarray(bk.reshape(NCHUNK, P).T),
        "bv": bv,
        "bo": bo,
    }
    in_maps = []
    for c in range(NB):
        # masksT[p, sc, ent] = entities[c, ent, sc*128 + p]
        m = entities[c].astype(np.float32)            # [EN, SL]
        mT = np.ascontiguousarray(
            m.reshape(EN, SCHUNK, P).transpose(2, 1, 0))
        in_maps.append({
            "toks": np.ascontiguousarray(tokens_embed[c]),
            "masksT": mT,
            **shared,
        })

    res = run_bass_kernel_spmd(nc, in_maps, core_ids=list(range(NB)))
    full = np.concatenate([res.results[c]["out"] for c in range(NB)], axis=0)
    # full[(b*EN + ent)*NE + e] = attention output for group (b, ent), event e

    # ragged selection (mirrors the reference indexing; identity for the
    # all-ones masks produced by setup_inputs)
    entity_index = np.flatnonzero(entity_masks.reshape(-1))
    pair_sel = (select_event[:, None, :] & entity_masks[:, :, None])
    pair_sel = pair_sel.reshape(-1, NE)[entity_index].reshape(-1)
    event_entity_index = np.flatnonzero(pair_sel)

    sel_rows = (entity_index[:, None] * NE + np.arange(NE)[None, :]).reshape(-1)
    return full[sel_rows][event_entity_index]


# revision 22
# speedup vs baseline: 31272.8811x; 31272.8811x over previous
"""EntityAttention Trainium2 kernel (nn_EntityAttention_31525059952740).

Math (per (batch, entity) group n, all 64 events e):
  q = (events @ Wq.T + bq) * scale            shared across n     [64, 512]
  k = toks_b @ Wk.T + bk                      per batch           [512, 512]
  v = toks_b @ Wv.T + bv                      per batch           [512, 512]
  scores[h,e,s] = q_h[e] . k_h[s]             per batch (2 heads x 256)
  attn = softmax over s, masked by entities[n]  (mask = multiplicative
         0/1 on exp since exp(-1e9 + x) == 0 in fp32)
  out[e] = concat_h(attn_h @ v_h);  O = out @ Wo.T + bo

Sharding: batch b -> core b (8 batches, 8 cores). Each core computes all 16
entities of its batch -> output rows [1024, 512] per core, concatenated.

Device layout ("transposed attention"): scores^T [s(partitions), (h,e)] so
that the entity mask is a per-partition scalar (fused into a single DVE
tensor_scalar per (entity, s-chunk)) and softmax denominators come from one
tiny PE matmul  S = masks^T.T @ exp(scores^T).

All heavy matmuls run as float32r (full PE rate, ~1e-4 relative error).
"""

import numpy as np

import concourse.bass as bass
import concourse.tile as tile
import concourse.mybir as mybir
from concourse import bacc
from concourse.bass_utils import run_bass_kernel_spmd

NB, SL, NH, EN, NE, HEADS = 8, 512, 512, 16, 64, 2
DH = NH // HEADS          # 256
P = 128
NCHUNK = NH // P          # 4 chunks of the hidden dim
SCHUNK = SL // P          # 4 chunks of the sequence dim
SCALE = 1.0 / np.sqrt(DH).astype(np.float32)

F32 = mybir.dt.float32
F32R = mybir.dt.float32r

_CACHE = {}


def _build():
    nc = bacc.Bacc("TRN2", target_bir_lowering=False, debug=False, num_devices=NB)

    # ---- I/O ----
    toksT_d = nc.dram_tensor("toksT", [NH, SL], F32R, kind="ExternalInput").ap()
    # packed small inputs: [128, 256 qT | 64 masksT | 4 bk | 512 bv | 128 ones | 512 bo]
    smalls_d = nc.dram_tensor("smalls", [P, 1476], F32R, kind="ExternalInput").ap()
    wkT_d = nc.dram_tensor("WkT", [NH, NH], F32R, kind="ExternalInput").ap()
    wvT_d = nc.dram_tensor("WvT", [NH, NH], F32R, kind="ExternalInput").ap()
    woT_d = nc.dram_tensor("WoT", [NH, NH], F32R, kind="ExternalInput").ap()
    out_d = nc.dram_tensor("out", [EN * NE, NH], F32, kind="ExternalOutput").ap()

    EXP = mybir.ActivationFunctionType.Exp
    CPY = mybir.ActivationFunctionType.Copy
    IDN = mybir.ActivationFunctionType.Identity

    with tile.TileContext(nc) as tc:
        with (
            tc.tile_pool(name="wpool", bufs=1) as wpool,
            tc.tile_pool(name="sb", bufs=1) as sb,
            tc.tile_pool(name="ostage", bufs=2) as ostage,
            tc.tile_pool(name="pbig", bufs=4, space="PSUM") as pbig,
            tc.tile_pool(name="psmall", bufs=4, space="PSUM") as psmall,
            tc.tile_pool(name="dram", bufs=1, space="DRAM") as dram,
        ):
            # ---------- loads ----------
            toksT_r = toksT_d.rearrange("(c p) s -> p c s", p=P)
            wk_r = wkT_d.rearrange("(c p) d -> p c d", p=P)
            wk_0 = wpool.tile([P, NCHUNK, P], F32R, tag="wk_0")
            nc.scalar.dma_start(wk_0[:], wk_r[:, :, :P])
            wk_1 = wpool.tile([P, NCHUNK, P], F32R, tag="wk_1")
            nc.scalar.dma_start(wk_1[:], wk_r[:, :, P:2 * P])
            toksT_t = []
            for hc in range(NCHUNK):
                t = sb.tile([P, SL], F32R, tag=f"toksT{hc}")
                nc.sync.dma_start(t[:], toksT_r[:, hc, :])
                toksT_t.append(t)
            wk_hi = wpool.tile([P, NCHUNK, 2 * P], F32R, tag="wk_hi")
            nc.scalar.dma_start(wk_hi[:], wk_r[:, :, 2 * P:])
            wv_sb = wpool.tile([P, NCHUNK, NH], F32R, tag="wv")
            nc.sync.dma_start(wv_sb[:], wvT_d.rearrange("(c p) d -> p c d", p=P))

            def toksT(hc):
                return toksT_t[hc][:]

            def wk_chunk(i, hc):
                if i < 2:
                    return (wk_0, wk_1)[i][:, hc, :]
                return wk_hi[:, hc, (i % 2) * P:(i % 2 + 1) * P]

            smalls = wpool.tile([P, 1476], F32R, tag="smalls")
            nc.scalar.dma_start(smalls[:], smalls_d)
            qT_sb = smalls[:, 0:256].rearrange("p (c e) -> p c e", c=NCHUNK)
            masks_sb = smalls[:, 256:320].rearrange("p (c e) -> p c e", c=SCHUNK)
            bk_sb = smalls[:, 320:324].bitcast(F32)
            bv_row = smalls[0:1, 324:836]
            ones_row = smalls[0:1, 836:964]
            bo_bc = smalls[:, 964:1476].bitcast(F32)

            wo_sb = wpool.tile([P, NCHUNK, NH], F32R, tag="wo")
            nc.sync.dma_start(wo_sb[:], woT_d.rearrange("(c p) d -> p c d", p=P))

            # ---------- K^T (V deferred: fills PE during attnT phase) -------
            kTs = []
            for i in range(NCHUNK):
                pk = pbig.tile([P, SL], F32, tag="pb", name=f"pk{i}")
                for hc in range(NCHUNK):
                    nc.tensor.matmul(
                        pk[:], wk_chunk(i, hc), toksT(hc),
                        start=(hc == 0), stop=(hc == NCHUNK - 1),
                    )
                kT = sb.tile([P, SL], F32R, tag=f"kT{i}")
                H = SL // 2
                if i % 2 == 0:
                    nc.scalar.activation(kT[:, :H], pk[:, :H], IDN,
                                         bias=bk_sb[:, i:i + 1])
                    nc.vector.tensor_scalar_add(kT[:, H:], pk[:, H:],
                                                bk_sb[:, i:i + 1])
                else:
                    nc.vector.tensor_scalar_add(kT[:, :H], pk[:, :H],
                                                bk_sb[:, i:i + 1])
                    nc.scalar.activation(kT[:, H:], pk[:, H:], IDN,
                                         bias=bk_sb[:, i:i + 1])
                kTs.append(kT)

            # ---------- scores^T -> E = exp(scores^T)  [s, (h,e)] ----------
            # h-major: head 0 only needs kT0/kT1, so PE starts ~1.3us earlier
            pss = [psmall.tile([P, HEADS * NE], F32, tag="psm", name=f"ps{sc}")
                   for sc in range(SCHUNK)]
            for h in range(HEADS):
                for sc in range(SCHUNK):
                    for j in range(2):
                        dc = 2 * h + j
                        nc.tensor.matmul(
                            pss[sc][:, h * NE:(h + 1) * NE],
                            kTs[dc][:, sc * P:(sc + 1) * P], qT_sb[:, dc, :],
                            start=(j == 0), stop=(j == 1),
                        )
            e_sbs = []
            for sc in range(SCHUNK):
                e_sb = sb.tile([P, HEADS * NE], F32R, tag=f"e{sc}")
                nc.scalar.activation(e_sb[:], pss[sc][:], EXP)
                e_sbs.append(e_sb)

            # ---------- S = masksT.T @ E ; recip; DRAM-roundtrip bcast -------
            pS = psmall.tile([EN, HEADS * NE], F32, tag="psm", name="pS")
            for sc in range(SCHUNK):
                nc.tensor.matmul(pS[:], masks_sb[:, sc, :], e_sbs[sc][:],
                                 start=(sc == 0), stop=(sc == SCHUNK - 1))
            srec = sb.tile([EN, HEADS * NE], F32, tag="srec")
            nc.vector.reciprocal(srec[:], pS[:])
            srec_dram = dram.tile([EN, HEADS * NE], F32)
            nc.sync.dma_start(srec_dram[:], srec[:])
            srec_bcs = []
            for grp in range(4):
                t = sb.tile([P, 4, HEADS * NE], F32, tag=f"srec_bc{grp}")
                sd_ap = srec_dram[grp * 4:(grp + 1) * 4, :]
                nc.sync.dma_start(
                    t[:],
                    bass.AP(tensor=sd_ap.tensor, offset=sd_ap.offset,
                            ap=[[0, P], *sd_ap.ap]),
                )
                srec_bcs.append(t)

            def srec_slice(grp, h):
                return srec_bcs[grp][:, :, h * NE:(h + 1) * NE]

            # ---------- V = toks @ WvT + bv (late PE filler) ----------
            vs = []
            for i in range(SCHUNK):
                pv = pbig.tile([P, NH], F32, tag="pb", name=f"pv{i}")
                for hc in range(NCHUNK):
                    nc.tensor.matmul(
                        pv[:], toksT(hc)[:, i * P:(i + 1) * P], wv_sb[:, hc, :],
                        start=(hc == 0), stop=False,
                    )
                nc.tensor.matmul(pv[:], ones_row, bv_row,
                                 start=False, stop=True)
                v = sb.tile([P, NH], F32R, tag=f"v{i}")
                H = NH // 2
                nc.scalar.activation(v[:, :H], pv[:, :H],
                                     mybir.ActivationFunctionType.Copy)
                nc.vector.tensor_copy(v[:, H:], pv[:, H:])
                vs.append(v)

            # ---------- attnT for all groups (4 groups x 4 entities) --------
            attnTs = {}
            for grp in range(4):
                for sc in range(SCHUNK):
                    attnT = sb.tile([P, 4, HEADS * NE], F32R,
                                    tag=f"attnT{grp}_{sc}")
                    for k in range(4):
                        ent = grp * 4 + k
                        if k < 2:
                            nc.vector.tensor_scalar_mul(
                                attnT[:, k, :], e_sbs[sc][:],
                                masks_sb[:, sc, ent:ent + 1].bitcast(F32),
                            )
                        elif k == 2:
                            nc.scalar.activation(
                                attnT[:, k, :], e_sbs[sc][:],
                                mybir.ActivationFunctionType.Copy,
                                scale=masks_sb[:, sc, ent:ent + 1].bitcast(F32),
                            )
                        else:
                            nc.gpsimd.tensor_scalar_mul(
                                attnT[:, k, :], e_sbs[sc][:],
                                masks_sb[:, sc, ent:ent + 1].bitcast(F32),
                            )
                    attnTs[(grp, sc)] = attnT

            # ---------- PV -> normalize -> O, per 4-entity group ----------
            for grp in range(4):
                outT = sb.tile([P, NCHUNK, 4, NE], F32R, tag=f"outT{grp}")
                for h in range(HEADS):
                    for j in range(2):
                        dc = 2 * h + j
                        po = pbig.tile([P, 4 * NE], F32, tag="pb",
                                       name=f"pos_{grp}_{dc}")
                        for sc in range(SCHUNK):
                            nc.tensor.matmul(
                                po[:],
                                vs[sc][:, dc * P:(dc + 1) * P],
                                attnTs[(grp, sc)][:, :, h * NE:(h + 1) * NE],
                                start=(sc == 0), stop=(sc == SCHUNK - 1),
                            )
                        nc.vector.tensor_mul(
                            outT[:, dc, :, :], po[:],
                            srec_slice(grp, h),
                        )
                o_sb = ostage.tile([P, 2, NH], F32)
                for lp in range(2):
                    pair = grp * 2 + lp
                    pO = pbig.tile([P, NH], F32, tag="pb", name=f"pO{pair}")
                    for hc in range(NCHUNK):
                        nc.tensor.matmul(
                            pO[:], outT[:, hc, 2 * lp:2 * lp + 2, :],
                            wo_sb[:, hc, :],
                            start=(hc == 0), stop=(hc == NCHUNK - 1),
                        )
                    nc.vector.tensor_add(o_sb[:, lp, :], pO[:], bo_bc)
                    if grp == 3:
                        nc.sync.dma_start(
                            out_d[pair * P:(pair + 1) * P, :], o_sb[:, lp, :])
                if grp < 3:
                    base = grp * 2 * P
                    nc.sync.dma_start(
                        out_d[base:base + 2 * P, :].rearrange(
                            "(q p) d -> p q d", p=P),
                        o_sb[:])

    nc.compile()
    return nc


def _get_nc():
    if "nc" not in _CACHE:
        _CACHE["nc"] = _build()
    return _CACHE["nc"]


def _fast_run(nc, in_maps):
    """Repeat-call path: same PJRT execution as run_bass_kernel_spmd/
    bass2jax.run_bass_via_pjrt, but with the jitted shard_map cached so
    repeat kernel() calls skip retracing/relowering."""
    import jax
    import jax.numpy as jnp
    from jax.sharding import Mesh, PartitionSpec
    from jax.experimental.shard_map import shard_map
    import concourse.mybir as mybir_
    from concourse import bass2jax

    if "runner" not in _CACHE:
        bass2jax.install_neuronx_cc_hook()
        part_name = (nc.partition_id_tensor.name
                     if nc.partition_id_tensor else None)
        in_names, out_names, out_avals = [], [], []
        for alloc in nc.m.functions[0].allocations:
            if not isinstance(alloc, mybir_.MemoryLocationSet):
                continue
            name = alloc.memorylocations[0].name
            if alloc.kind == "ExternalInput":
                if name != part_name:
                    in_names.append(name)
            elif alloc.kind == "ExternalOutput":
                out_names.append(name)
                out_avals.append(jax.core.ShapedArray(
                    tuple(alloc.tensor_shape), mybir_.dt.np(alloc.dtype)))
        n_params = len(in_names)
        all_in_names = in_names + out_names
        if part_name is not None:
            all_in_names = all_in_names + [part_name]

        def _body(*args):
            operands = list(args)
            if part_name is not None:
                operands.append(bass2jax.partition_id_tensor())
            outs = bass2jax._bass_exec_p.bind(
                *operands,
                out_avals=tuple(out_avals),
                in_names=tuple(all_in_names),
                out_names=tuple(out_names),
                lowering_input_output_aliases=(),
                sim_require_finite=True,
                sim_require_nnan=True,
                nc=nc,
            )
            return tuple(outs)

        devices = jax.devices()[:NB]
        mesh = Mesh(np.asarray(devices), ("core",))
        n_outs = len(out_names)
        sharded = jax.jit(
            shard_map(_body, mesh=mesh,
                      in_specs=(PartitionSpec("core"),) * (n_params + n_outs),
                      out_specs=(PartitionSpec("core"),) * n_outs,
                      check_rep=False),
            donate_argnums=tuple(range(n_params, n_params + n_outs)),
            keep_unused=True,
        )
        _CACHE["runner"] = (sharded, in_names, out_names, out_avals)

    sharded, in_names, out_names, out_avals = _CACHE["runner"]
    concat_in = [
        np.concatenate([np.asarray(m[name]) for m in in_maps], axis=0)
        for name in in_names
    ]
    concat_zeros = [
        np.zeros((NB * av.shape[0], *av.shape[1:]), av.dtype)
        for av in out_avals
    ]
    out_arrs = sharded(*concat_in, *concat_zeros)
    return [
        {name: np.asarray(out_arrs[i]).reshape(NB, *out_avals[i].shape)[c]
         for i, name in enumerate(out_names)}
        for c in range(NB)
    ]


def kernel(tokens_embed, entities, events_embed, entity_num, entity_masks,
           select_event, Wq, Wk, Wv, bq, bk, bv, Wo, bo):
    tokens_embed = np.asarray(tokens_embed, dtype=np.float32)
    entities = np.asarray(entities)
    events_embed = np.asarray(events_embed, dtype=np.float32)
    entity_masks = np.asarray(entity_masks)
    select_event = np.asarray(select_event)
    Wq = np.asarray(Wq, dtype=np.float32)
    Wk = np.asarray(Wk, dtype=np.float32)
    Wv = np.asarray(Wv, dtype=np.float32)
    Wo = np.asarray(Wo, dtype=np.float32)
    bq = np.asarray(bq, dtype=np.float32)
    bk = np.asarray(bk, dtype=np.float32)
    bv = np.asarray(bv, dtype=np.float32)
    bo = np.asarray(bo, dtype=np.float32)

    nc = _get_nc()

    q_s = (events_embed @ Wq.T + bq) * SCALE          # [NE, NH]
    qT_pc = q_s.T.reshape(NCHUNK, P, NE).transpose(1, 0, 2).reshape(P, -1)
    smalls = np.zeros((P, 1476), dtype=np.float32)
    smalls[:, 0:256] = qT_pc
    smalls[:, 320:324] = bk.reshape(NCHUNK, P).T
    smalls[:, 324:836] = bv[None, :]
    smalls[:, 836:964] = 1.0
    smalls[:, 964:1476] = bo[None, :]
    shared = {
        "WkT": np.ascontiguousarray(Wk.T),
        "WvT": np.ascontiguousarray(Wv.T),
        "WoT": np.ascontiguousarray(Wo.T),
    }
    in_maps = []
    for c in range(NB):
        # masksT[p, sc, ent] = entities[c, ent, sc*128 + p]
        m = entities[c].astype(np.float32)            # [EN, SL]
        mT = m.reshape(EN, SCHUNK, P).transpose(2, 1, 0).reshape(P, -1)
        sm = smalls.copy()
        sm[:, 256:320] = mT
        in_maps.append({
            "toksT": np.ascontiguousarray(tokens_embed[c].T),
            "smalls": sm,
            **shared,
        })

    if "ran_once" not in _CACHE:
        res = run_bass_kernel_spmd(nc, in_maps, core_ids=list(range(NB)))
        results = res.results
        _CACHE["ran_once"] = True
    else:
        results = _fast_run(nc, in_maps)
    full = np.concatenate([results[c]["out"] for c in range(NB)], axis=0)
    # full[(b*EN + ent)*NE + e] = attention output for group (b, ent), event e

    # ragged selection (mirrors the reference indexing; identity for the
    # all-ones masks produced by setup_inputs)
    assert int(entity_num) == EN
    entity_index = np.flatnonzero(entity_masks.reshape(-1))
    pair_sel = (select_event[:, None, :] & entity_masks[:, :, None])
    pair_sel = pair_sel.reshape(-1, NE)[entity_index].reshape(-1)
    event_entity_index = np.flatnonzero(pair_sel)

    sel_rows = (entity_index[:, None] * NE + np.arange(NE)[None, :]).reshape(-1)
    return full[sel_rows][event_entity_index]



# revision 28
# speedup vs baseline: 31312.2508x; 1.0013x over previous
"""EntityAttention Trainium2 kernel (nn_EntityAttention_31525059952740).

Math (per (batch, entity) group n, all 64 events e):
  q = (events @ Wq.T + bq) * scale            shared across n     [64, 512]
  k = toks_b @ Wk.T + bk                      per batch           [512, 512]
  v = toks_b @ Wv.T + bv                      per batch           [512, 512]
  scores[h,e,s] = q_h[e] . k_h[s]             per batch (2 heads x 256)
  attn = softmax over s, masked by entities[n]  (mask = multiplicative
         0/1 on exp since exp(-1e9 + x) == 0 in fp32)
  out[e] = concat_h(attn_h @ v_h);  O = out @ Wo.T + bo

Sharding: batch b -> core b (8 batches, 8 cores). Each core computes all 16
entities of its batch -> output rows [1024, 512] per core, concatenated.

Device layout ("transposed attention"): scores^T [s(partitions), (h,e)] so
that the entity mask is a per-partition scalar (fused into a single DVE
tensor_scalar per (entity, s-chunk)) and softmax denominators come from one
tiny PE matmul  S = masks^T.T @ exp(scores^T).

All heavy matmuls run as float32r (full PE rate, ~1e-4 relative error).
"""

import numpy as np

import concourse.bass as bass
import concourse.tile as tile
import concourse.mybir as mybir
from concourse import bacc
from concourse.bass_utils import run_bass_kernel_spmd

NB, SL, NH, EN, NE, HEADS = 8, 512, 512, 16, 64, 2
DH = NH // HEADS          # 256
P = 128
NCHUNK = NH // P          # 4 chunks of the hidden dim
SCHUNK = SL // P          # 4 chunks of the sequence dim
SCALE = 1.0 / np.sqrt(DH).astype(np.float32)

F32 = mybir.dt.float32
F32R = mybir.dt.float32r

_CACHE = {}


def _build():
    nc = bacc.Bacc("TRN2", target_bir_lowering=False, debug=False, num_devices=NB)

    # ---- I/O ----
    toksT_d = nc.dram_tensor("toksT", [NH, SL], F32R, kind="ExternalInput").ap()
    # packed small inputs: [128, 256 qT | 64 masksT | 4 bk | 512 bv | 128 ones | 512 bo]
    smalls_d = nc.dram_tensor("smalls", [P, 1476], F32R, kind="ExternalInput").ap()
    wkT_d = nc.dram_tensor("WkT", [NH, NH], F32R, kind="ExternalInput").ap()
    wvT_d = nc.dram_tensor("WvT", [NH, NH], F32R, kind="ExternalInput").ap()
    woT_d = nc.dram_tensor("WoT", [NH, NH], F32R, kind="ExternalInput").ap()
    out_d = nc.dram_tensor("out", [EN * NE, NH], F32, kind="ExternalOutput").ap()

    EXP = mybir.ActivationFunctionType.Exp
    CPY = mybir.ActivationFunctionType.Copy
    IDN = mybir.ActivationFunctionType.Identity

    with tile.TileContext(nc) as tc:
        with (
            tc.tile_pool(name="wpool", bufs=1) as wpool,
            tc.tile_pool(name="sb", bufs=1) as sb,
            tc.tile_pool(name="ostage", bufs=2) as ostage,
            tc.tile_pool(name="pbig", bufs=4, space="PSUM") as pbig,
            tc.tile_pool(name="psmall", bufs=4, space="PSUM") as psmall,
            tc.tile_pool(name="dram", bufs=1, space="DRAM") as dram,
        ):
            # ---------- loads ----------
            toksT_r = toksT_d.rearrange("(c p) s -> p c s", p=P)
            wk_r = wkT_d.rearrange("(c p) d -> p c d", p=P)
            wk_0 = wpool.tile([P, NCHUNK, P], F32R, tag="wk_0")
            nc.scalar.dma_start(wk_0[:], wk_r[:, :, :P])
            wk_1 = wpool.tile([P, NCHUNK, P], F32R, tag="wk_1")
            nc.scalar.dma_start(wk_1[:], wk_r[:, :, P:2 * P])
            toksT_t = []
            for hc in range(NCHUNK):
                t = sb.tile([P, SL], F32R, tag=f"toksT{hc}")
                nc.sync.dma_start(t[:], toksT_r[:, hc, :])
                toksT_t.append(t)
            wk_hi = wpool.tile([P, NCHUNK, 2 * P], F32R, tag="wk_hi")
            nc.scalar.dma_start(wk_hi[:], wk_r[:, :, 2 * P:])
            wv_sb = wpool.tile([P, NCHUNK, NH], F32R, tag="wv")
            nc.sync.dma_start(wv_sb[:], wvT_d.rearrange("(c p) d -> p c d", p=P))

            def toksT(hc):
                return toksT_t[hc][:]

            def wk_chunk(i, hc):
                if i < 2:
                    return (wk_0, wk_1)[i][:, hc, :]
                return wk_hi[:, hc, (i % 2) * P:(i % 2 + 1) * P]

            smalls = wpool.tile([P, 1476], F32R, tag="smalls")
            nc.scalar.dma_start(smalls[:], smalls_d)
            qT_sb = smalls[:, 0:256].rearrange("p (c e) -> p c e", c=NCHUNK)
            masks_sb = smalls[:, 256:320].rearrange("p (c e) -> p c e", c=SCHUNK)
            bk_sb = smalls[:, 320:324].bitcast(F32)
            bv_row = smalls[0:1, 324:836]
            ones_row = smalls[0:1, 836:964]
            bo_bc = smalls[:, 964:1476].bitcast(F32)

            wo_sb = wpool.tile([P, NCHUNK, NH], F32R, tag="wo")
            nc.sync.dma_start(wo_sb[:], woT_d.rearrange("(c p) d -> p c d", p=P))

            # ---------- K^T (V deferred: fills PE during attnT phase) -------
            kTs = []
            for i in range(NCHUNK):
                pk = pbig.tile([P, SL], F32, tag="pb", name=f"pk{i}")
                for hc in range(NCHUNK):
                    nc.tensor.matmul(
                        pk[:], wk_chunk(i, hc), toksT(hc),
                        start=(hc == 0), stop=(hc == NCHUNK - 1),
                    )
                kT = sb.tile([P, SL], F32R, tag=f"kT{i}")
                H = SL // 2
                if i % 2 == 0:
                    nc.scalar.activation(kT[:, :H], pk[:, :H], IDN,
                                         bias=bk_sb[:, i:i + 1])
                    nc.vector.tensor_scalar_add(kT[:, H:], pk[:, H:],
                                                bk_sb[:, i:i + 1])
                else:
                    nc.vector.tensor_scalar_add(kT[:, :H], pk[:, :H],
                                                bk_sb[:, i:i + 1])
                    nc.scalar.activation(kT[:, H:], pk[:, H:], IDN,
                                         bias=bk_sb[:, i:i + 1])
                kTs.append(kT)

            # ---------- scores^T -> E = exp(scores^T)  [s, (h,e)] ----------
            # h-major: head 0 only needs kT0/kT1, so PE starts ~1.3us earlier
            pss = [psmall.tile([P, HEADS * NE], F32, tag="psm", name=f"ps{sc}")
                   for sc in range(SCHUNK)]
            for h in range(HEADS):
                for sc in range(SCHUNK):
                    for j in range(2):
                        dc = 2 * h + j
                        nc.tensor.matmul(
                            pss[sc][:, h * NE:(h + 1) * NE],
                            kTs[dc][:, sc * P:(sc + 1) * P], qT_sb[:, dc, :],
                            start=(j == 0), stop=(j == 1),
                        )
            e_sbs = []
            for sc in range(SCHUNK):
                e_sb = sb.tile([P, HEADS * NE], F32R, tag=f"e{sc}")
                nc.scalar.activation(e_sb[:], pss[sc][:], EXP)
                e_sbs.append(e_sb)

            # ---------- S = masksT.T @ E ; recip; DRAM-roundtrip bcast -------
            pS = psmall.tile([EN, HEADS * NE], F32, tag="psm", name="pS")
            for sc in range(SCHUNK):
                nc.tensor.matmul(pS[:], masks_sb[:, sc, :], e_sbs[sc][:],
                                 start=(sc == 0), stop=(sc == SCHUNK - 1))
            srec = sb.tile([EN, HEADS * NE], F32, tag="srec")
            nc.vector.reciprocal(srec[:], pS[:])
            srec_dram = dram.tile([EN, HEADS * NE], F32)
            nc.sync.dma_start(srec_dram[:], srec[:])
            srec_bcs = []
            for grp in range(4):
                t = sb.tile([P, 4, HEADS * NE], F32, tag=f"srec_bc{grp}")
                sd_ap = srec_dram[grp * 4:(grp + 1) * 4, :]
                nc.sync.dma_start(
                    t[:],
                    bass.AP(tensor=sd_ap.tensor, offset=sd_ap.offset,
                            ap=[[0, P], *sd_ap.ap]),
                )
                srec_bcs.append(t)

            def srec_slice(grp, h):
                return srec_bcs[grp][:, :, h * NE:(h + 1) * NE]

            # ---------- V = toks @ WvT + bv (late PE filler) ----------
            vs = []
            for i in range(SCHUNK):
                pv = pbig.tile([P, NH], F32, tag="pb", name=f"pv{i}")
                for hc in range(NCHUNK):
                    nc.tensor.matmul(
                        pv[:], toksT(hc)[:, i * P:(i + 1) * P], wv_sb[:, hc, :],
                        start=(hc == 0), stop=False,
                    )
                nc.tensor.matmul(pv[:], ones_row, bv_row,
                                 start=False, stop=True)
                v = sb.tile([P, NH], F32R, tag=f"v{i}")
                H = NH // 2
                nc.scalar.activation(v[:, :H], pv[:, :H],
                                     mybir.ActivationFunctionType.Copy)
                nc.vector.tensor_copy(v[:, H:], pv[:, H:])
                vs.append(v)

            # ---------- attnT for all groups (4 groups x 4 entities) --------
            attnTs = {}
            for grp in range(4):
                for sc in range(SCHUNK):
                    attnT = sb.tile([P, 4, HEADS * NE], F32R,
                                    tag=f"attnT{grp}_{sc}")
                    for k in range(4):
                        ent = grp * 4 + k
                        if k < 2:
                            nc.vector.tensor_scalar_mul(
                                attnT[:, k, :], e_sbs[sc][:],
                                masks_sb[:, sc, ent:ent + 1].bitcast(F32),
                            )
                        elif k == 2:
                            nc.scalar.activation(
                                attnT[:, k, :], e_sbs[sc][:],
                                mybir.ActivationFunctionType.Copy,
                                scale=masks_sb[:, sc, ent:ent + 1].bitcast(F32),
                            )
                        else:
                            nc.gpsimd.tensor_scalar_mul(
                                attnT[:, k, :], e_sbs[sc][:],
                                masks_sb[:, sc, ent:ent + 1].bitcast(F32),
                            )
                    attnTs[(grp, sc)] = attnT

            # ---------- PV -> normalize -> O, per 4-entity group ----------
            for grp in range(4):
                outT = sb.tile([P, NCHUNK, 4, NE], F32R, tag=f"outT{grp}")
                for h in range(HEADS):
                    for j in range(2):
                        dc = 2 * h + j
                        po = pbig.tile([P, 4 * NE], F32, tag="pb",
                                       name=f"pos_{grp}_{dc}")
                        for sc in range(SCHUNK):
                            nc.tensor.matmul(
                                po[:],
                                vs[sc][:, dc * P:(dc + 1) * P],
                                attnTs[(grp, sc)][:, :, h * NE:(h + 1) * NE],
                                start=(sc == 0), stop=(sc == SCHUNK - 1),
                            )
                        nc.vector.tensor_mul(
                            outT[:, dc, :, :], po[:],
                            srec_slice(grp, h),
                        )
                o_sb = ostage.tile([P, 2, NH], F32)
                for lp in range(2):
                    pair = grp * 2 + lp
                    pO = pbig.tile([P, NH], F32, tag="pb", name=f"pO{pair}")
                    for hc in range(NCHUNK):
                        nc.tensor.matmul(
                            pO[:], outT[:, hc, 2 * lp:2 * lp + 2, :],
                            wo_sb[:, hc, :],
                            start=(hc == 0), stop=(hc == NCHUNK - 1),
                        )
                    nc.vector.tensor_add(o_sb[:, lp, :], pO[:], bo_bc)
                    if grp >= 2:
                        nc.sync.dma_start(
                            out_d[pair * P:(pair + 1) * P, :], o_sb[:, lp, :])
                if grp < 2:
                    base = grp * 2 * P
                    nc.sync.dma_start(
                        out_d[base:base + 2 * P, :].rearrange(
                            "(q p) d -> p q d", p=P),
                        o_sb[:])

    nc.compile()
    return nc


def _get_nc():
    if "nc" not in _CACHE:
        _CACHE["nc"] = _build()
    return _CACHE["nc"]


def _fast_run(nc, in_maps):
    """Repeat-call path: same PJRT execution as run_bass_kernel_spmd/
    bass2jax.run_bass_via_pjrt, but with the jitted shard_map cached so
    repeat kernel() calls skip retracing/relowering."""
    import jax
    import jax.numpy as jnp
    from jax.sharding import Mesh, PartitionSpec
    from jax.experimental.shard_map import shard_map
    import concourse.mybir as mybir_
    from concourse import bass2jax

    if "runner" not in _CACHE:
        bass2jax.install_neuronx_cc_hook()
        part_name = (nc.partition_id_tensor.name
                     if nc.partition_id_tensor else None)
        in_names, out_names, out_avals = [], [], []
        for alloc in nc.m.functions[0].allocations:
            if not isinstance(alloc, mybir_.MemoryLocationSet):
                continue
            name = alloc.memorylocations[0].name
            if alloc.kind == "ExternalInput":
                if name != part_name:
                    in_names.append(name)
            elif alloc.kind == "ExternalOutput":
                out_names.append(name)
                out_avals.append(jax.core.ShapedArray(
                    tuple(alloc.tensor_shape), mybir_.dt.np(alloc.dtype)))
        n_params = len(in_names)
        all_in_names = in_names + out_names
        if part_name is not None:
            all_in_names = all_in_names + [part_name]

        def _body(*args):
            operands = list(args)
            if part_name is not None:
                operands.append(bass2jax.partition_id_tensor())
            outs = bass2jax._bass_exec_p.bind(
                *operands,
                out_avals=tuple(out_avals),
                in_names=tuple(all_in_names),
                out_names=tuple(out_names),
                lowering_input_output_aliases=(),
                sim_require_finite=True,
                sim_require_nnan=True,
                nc=nc,
            )
            return tuple(outs)

        devices = jax.devices()[:NB]
        mesh = Mesh(np.asarray(devices), ("core",))
        n_outs = len(out_names)
        sharded = jax.jit(
            shard_map(_body, mesh=mesh,
                      in_specs=(PartitionSpec("core"),) * (n_params + n_outs),
                      out_specs=(PartitionSpec("core"),) * n_outs,
                      check_rep=False),
            donate_argnums=tuple(range(n_params, n_params + n_outs)),
            keep_unused=True,
        )
        _CACHE["runner"] = (sharded, in_names, out_names, out_avals)

    sharded, in_names, out_names, out_avals = _CACHE["runner"]
    concat_in = [
        np.concatenate([np.asarray(m[name]) for m in in_maps], axis=0)
        for name in in_names
    ]
    concat_zeros = [
        np.zeros((NB * av.shape[0], *av.shape[1:]), av.dtype)
        for av in out_avals
    ]
    out_arrs = sharded(*concat_in, *concat_zeros)
    return [
        {name: np.asarray(out_arrs[i]).reshape(NB, *out_avals[i].shape)[c]
         for i, name in enumerate(out_names)}
        for c in range(NB)
    ]


def kernel(tokens_embed, entities, events_embed, entity_num, entity_masks,
           select_event, Wq, Wk, Wv, bq, bk, bv, Wo, bo):
    tokens_embed = np.asarray(tokens_embed, dtype=np.float32)
    entities = np.asarray(entities)
    events_embed = np.asarray(events_embed, dtype=np.float32)
    entity_masks = np.asarray(entity_masks)
    select_event = np.asarray(select_event)
    Wq = np.asarray(Wq, dtype=np.float32)
    Wk = np.asarray(Wk, dtype=np.float32)
    Wv = np.asarray(Wv, dtype=np.float32)
    Wo = np.asarray(Wo, dtype=np.float32)
    bq = np.asarray(bq, dtype=np.float32)
    bk = np.asarray(bk, dtype=np.float32)
    bv = np.asarray(bv, dtype=np.float32)
    bo = np.asarray(bo, dtype=np.float32)

    nc = _get_nc()

    q_s = (events_embed @ Wq.T + bq) * SCALE          # [NE, NH]
    qT_pc = q_s.T.reshape(NCHUNK, P, NE).transpose(1, 0, 2).reshape(P, -1)
    smalls = np.zeros((P, 1476), dtype=np.float32)
    smalls[:, 0:256] = qT_pc
    smalls[:, 320:324] = bk.reshape(NCHUNK, P).T
    smalls[:, 324:836] = bv[None, :]
    smalls[:, 836:964] = 1.0
    smalls[:, 964:1476] = bo[None, :]
    shared = {
        "WkT": np.ascontiguousarray(Wk.T),
        "WvT": np.ascontiguousarray(Wv.T),
        "WoT": np.ascontiguousarray(Wo.T),
    }
    in_maps = []
    for c in range(NB):
        # masksT[p, sc, ent] = entities[c, ent, sc*128 + p]
        m = entities[c].astype(np.float32)            # [EN, SL]
        mT = m.reshape(EN, SCHUNK, P).transpose(2, 1, 0).reshape(P, -1)
        sm = smalls.copy()
        sm[:, 256:320] = mT
        in_maps.append({
            "toksT": np.ascontiguousarray(tokens_embed[c].T),
            "smalls": sm,
            **shared,
        })

    if "ran_once" not in _CACHE:
        res = run_bass_kernel_spmd(nc, in_maps, core_ids=list(range(NB)))
        results = res.results
        _CACHE["ran_once"] = True
    else:
        results = _fast_run(nc, in_maps)
    full = np.concatenate([results[c]["out"] for c in range(NB)], axis=0)
    # full[(b*EN + ent)*NE + e] = attention output for group (b, ent), event e

    # ragged selection (mirrors the reference indexing; identity for the
    # all-ones masks produced by setup_inputs)
    assert int(entity_num) == EN
    entity_index = np.flatnonzero(entity_masks.reshape(-1))
    pair_sel = (select_event[:, None, :] & entity_masks[:, :, None])
    pair_sel = pair_sel.reshape(-1, NE)[entity_index].reshape(-1)
    event_entity_index = np.flatnonzero(pair_sel)

    sel_rows = (entity_index[:, None] * NE + np.arange(NE)[None, :]).reshape(-1)
    return full[sel_rows][event_entity_index]

